# revision 1
# baseline (speedup 1.0000x reference)
"""Chebyshev encoder kernel for Trainium2 (8 NeuronCores, SPMD data-parallel).

Input : x (500000, 2) float32 in [0,1)
Output: (500000, 256) float32 where out[n, d*128 + k] = T_k(x[n, d])

Strategy
--------
Flatten x to 1M elements, shard 125k elements per core (row-sharding of N).
Per core, elements live in a [128, 977] layout (56 padded elements). The
output per element is its 128 Chebyshev values stored contiguously, so the
per-core output is [128, 977*128] with a fully contiguous per-partition DMA.

Compute per chunk of F elements-per-partition, into an SBUF tile whose free
dim interleaves (f, k) as f*128+k:
  * cols 0/1: ones and x (ScalarE copies)
  * cols 2..8: plain recurrence T_k = 2x*T_{k-1} - T_{k-2}
      (VectorE tensor_mul + fused scalar_tensor_tensor)
  * cols 9..127 by doubling rounds m = 8,16,32,64:
      odd  j: T_{m+j} = (T_m*T_j)*2 - T_{m-j}   -> VectorE mult + STT,
              one multi-column strided-AP op per round (T_m broadcast)
      even j: T_{2n} = Square(sqrt(2)*T_n) - 1  -> ScalarE only
This splits the elementwise work across VectorE and ScalarE and keeps the
kernel DMA-bound (~64MB of output per core).
"""

import numpy as np

N_ROWS = 500000
D = 2
ORDER = 128
NCORES = 8
EPC = N_ROWS * D // NCORES      # 125000 elements per core
PPART = 128                     # SBUF partitions
TPP = 977                       # elements per partition (128*977 = 125056)
PAD = PPART * TPP - EPC         # 56
CHUNK_FS = [125] * 7 + [102]    # sum = 977
BASE = 8                        # plain-recurrence depth (accuracy)
SQRT2 = float(np.sqrt(2.0))

_CACHE = {}


def build_bass(reps: int = 1):
    """Build the (finalized) single-core SPMD Bass program."""
    import concourse.bacc as bacc
    import concourse.mybir as mybir
    import concourse.tile as tile

    nc = bacc.Bacc("TRN2", target_bir_lowering=False, debug=False)
    f32 = mybir.dt.float32
    x_dram = nc.dram_tensor("x", [PPART, TPP], f32, kind="ExternalInput")
    out_dram = nc.dram_tensor("out", [PPART, TPP * ORDER], f32, kind="ExternalOutput")

    with tile.TileContext(nc) as tc:
        with (
            tc.tile_pool(name="io", bufs=2) as iop,
            tc.tile_pool(name="cst", bufs=1) as cstp,
            tc.tile_pool(name="tmp", bufs=1) as tpool,
        ):
            x_t = cstp.tile([PPART, TPP], f32, tag="x")
            nc.sync.dma_start(out=x_t[:], in_=x_dram[:])
            neg1 = cstp.tile([PPART, 1], f32, tag="neg1")
            nc.vector.memset(neg1[:], -1.0)

            for _ in range(reps):
                f0 = 0
                for F in CHUNK_FS:
                    xs = x_t[:, f0:f0 + F]
                    ot = iop.tile([PPART, CHUNK_FS[0] * ORDER], f32, tag="ot")
                    o3 = ot[:, : F * ORDER].rearrange("p (f k) -> p f k", k=ORDER)

                    # col 1 = x, col 0 = 1
                    nc.scalar.copy(out=o3[:, :, 1], in_=xs)
                    nc.scalar.activation(
                        out=o3[:, :, 0], in_=xs,
                        func=mybir.ActivationFunctionType.Copy,
                        bias=1.0, scale=0.0)

                    # base recurrence
                    mt = tpool.tile([PPART, CHUNK_FS[0]], f32, tag="mt")
                    for k in range(2, BASE + 1):
                        nc.vector.tensor_mul(
                            out=mt[:, :F], in0=xs, in1=o3[:, :, k - 1])
                        nc.vector.scalar_tensor_tensor(
                            out=o3[:, :, k], in0=mt[:, :F], scalar=2.0,
                            in1=o3[:, :, k - 2],
                            op0=mybir.AluOpType.mult,
                            op1=mybir.AluOpType.subtract)

                    # doubling rounds
                    m = BASE
                    while m < ORDER - 1:
                        njo = m // 2                    # odd j count
                        mo = tpool.tile(
                            [PPART, CHUNK_FS[0] * (ORDER // 4)], f32, tag="mo")
                        mo3 = mo[:, : F * njo].rearrange(
                            "p (f j) -> p f j", j=njo)
                        nc.vector.tensor_mul(
                            out=mo3,
                            in0=o3[:, :, m:m + 1].broadcast_to([PPART, F, njo]),
                            in1=o3[:, :, 1:m:2])
                        nc.vector.scalar_tensor_tensor(
                            out=o3[:, :, m + 1:2 * m:2], in0=mo3, scalar=2.0,
                            in1=o3[:, :, m - 1:0:-2],
                            op0=mybir.AluOpType.mult,
                            op1=mybir.AluOpType.subtract)

                        top_col = min(2 * m, ORDER - 2)  # last even col
                        nje = (top_col - (m + 2)) // 2 + 1
                        n_lo = m // 2 + 1
                        sq = tpool.tile(
                            [PPART, CHUNK_FS[0] * (ORDER // 4)], f32, tag="sq")
                        sq3 = sq[:, : F * nje].rearrange(
                            "p (f j) -> p f j", j=nje)
                        nc.scalar.activation(
                            out=sq3, in_=o3[:, :, n_lo:n_lo + nje],
                            func=mybir.ActivationFunctionType.Square,
                            scale=SQRT2)
                        nc.scalar.activation(
                            out=o3[:, :, m + 2:m + 2 + 2 * nje:2], in_=sq3,
                            func=mybir.ActivationFunctionType.Identity,
                            bias=neg1[:], scale=1.0)
                        m *= 2

                    nc.sync.dma_start(
                        out=out_dram[:, f0 * ORDER:(f0 + F) * ORDER],
                        in_=ot[:, : F * ORDER])
                    f0 += F

    nc.finalize()
    return nc


def make_in_maps(x: np.ndarray):
    flat = np.ascontiguousarray(x, dtype=np.float32).reshape(-1)
    in_maps = []
    for c in range(NCORES):
        shard = flat[c * EPC:(c + 1) * EPC]
        shard = np.concatenate([shard, np.zeros(PAD, np.float32)])
        in_maps.append({"x": shard.reshape(PPART, TPP)})
    return in_maps


def assemble(results) -> np.ndarray:
    parts = [
        np.asarray(results[c]["out"]).reshape(PPART * TPP, ORDER)[:EPC]
        for c in range(NCORES)
    ]
    full = np.concatenate(parts, axis=0)          # (1000000, 128)
    return np.ascontiguousarray(full).reshape(N_ROWS, D * ORDER)


def kernel(x: np.ndarray) -> np.ndarray:
    from concourse.bass_utils import run_bass_kernel_spmd

    if "nc" not in _CACHE:
        _CACHE["nc"] = build_bass(reps=1)
    nc = _CACHE["nc"]
    res = run_bass_kernel_spmd(nc, make_in_maps(x), core_ids=list(range(NCORES)))
    return assemble(res.results)
